# revision 4
# baseline (speedup 1.0000x reference)
"""Trainium2 Bass kernel for nn_MergePooling (segment mean/max pooling with a
gated linear combine), distributed over 8 NeuronCores.

Sharding: segment-aligned - core m owns segments [1024m, 1024(m+1)) and the
corresponding (sorted) node rows, so no cross-core collective is needed.

v2 design ("fold" layout):
- x is cast to bf16 on the host (rel-err budget 2e-2; bf16 adds ~3e-3),
  halving HBM traffic. Rows are padded so every segment covers whole 16-row
  groups, then swizzled so that within each 2048-row "fold set" the 16 rows
  of group slot g live at partition g of 16 consecutive 128x128 subtiles.
- Group max: 4 log2 tensor_tensor(max) folds on the raw bf16 SBUF tiles
  (DVE 2x mode, no PSUM reads) -> one [group, ch] tile per fold set.
- Group sum: 16 transposing matmuls (lhsT = subtile, rhs = I128) accumulated
  into one PSUM tile -> exact f32 sums already transposed to [ch, group].
  The fold-max tile takes one more transposing matmul.
- ACT copies both PSUM tiles out to the [ch, group] sumg/maxg arrays; two
  masked segmented scans (DVE) + a gpsimd gather produce per-segment
  sums/maxes; the tiny gated combine runs on PE/ACT/DVE as before.
"""

import numpy as np
import ml_dtypes

import bass_rust
import concourse.bass as bass
import concourse.mybir as mybir
import concourse.tile as tile
from concourse.bass_utils import run_bass_kernel_spmd
import concourse.bass_utils as _bu

# birsim (the C++ BIR simulator walrus runs at compile time) takes many
# minutes on a ~3k-instruction kernel; disable it for this compile.
_orig_bvo = _bu.bir_verify_and_optimise
def _bvo_fast(tmpdir, inp="bir.json", outp="file.neff", arch=None, *, dve_root=None):
    _orig_run = _bu.run_command
    def _patched_run(cmd, cwd=None):
        cmd = [c.replace("--enable-birsim=true", "--enable-birsim=false") for c in cmd]
        return _orig_run(cmd, cwd=cwd)
    _bu.run_command = _patched_run
    try:
        return _orig_bvo(tmpdir, inp, outp, arch, dve_root=dve_root)
    finally:
        _bu.run_command = _orig_run
_bu.bir_verify_and_optimise = _bvo_fast

P = 128            # partitions
C = 128            # feature channels
N_GRAPHS = 8192
CORES = 8
SEG_PER_CORE = N_GRAPHS // CORES   # 1024
GROUP = 16         # node rows per level-1 group (segments padded to this)
FSET = 2048        # rows per fold set (128 group slots x 16 rows)
SETS_PER_DMA = 4
DBLK = FSET * SETS_PER_DMA         # 8192 rows per DMA block
CHUNK = 1024       # groups per level-2 scan chunk
F32 = mybir.dt.float32
BF16 = mybir.dt.bfloat16
NPBF16 = ml_dtypes.bfloat16


def _split_multi_waits(nc):
    """This walrus build accepts a single sync-wait per instruction; Tile can
    attach several. Move extras onto preceding same-engine NoOp waits."""
    ctr = 0
    for f in nc.m.functions:
        for bb in f.blocks:
            out, dirty = [], False
            for inst in bb.instructions:
                si = inst.sync_info
                if si is not None and si.on_wait is not None and len(si.on_wait) > 1:
                    waits = list(si.on_wait)
                    for w in waits[:-1]:
                        ctr += 1
                        out.append(bass_rust.InstNoOp(
                            name=f"waitsplit-{ctr}",
                            engine=inst.engine,
                            ins=[], outs=[],
                            sync_info=mybir.SyncInfo(on_update=[], on_wait=[w]),
                        ))
                    si.on_wait = waits[-1:]
                    dirty = True
                out.append(inst)
            if dirty:
                bb.instructions = out


def _build_program(NPB, NG):
    """One SPMD program; all shapes identical across cores."""
    NSETS = NPB // FSET
    NB = NPB // DBLK
    NCH = (NG + CHUNK - 1) // CHUNK
    assert NG % 512 == 0 and NPB % DBLK == 0
    GPS = FSET // GROUP  # 128 group slots per fold set

    nc = bass.Bass("TRN2", target_bir_lowering=False, debug=False)
    xp_h = nc.declare_dram_parameter("xp", [NB * P, SETS_PER_DMA * GROUP * C], BF16,
                                     isOutput=False)
    ma_h = nc.declare_dram_parameter("mask_add", [1, NG], BF16, isOutput=False)
    mm_h = nc.declare_dram_parameter("mask_mul", [1, NG], BF16, isOutput=False)
    gi_h = nc.declare_dram_parameter("gidx", [P, SEG_PER_CORE // 16], mybir.dt.uint16,
                                     isOutput=False)
    rc_h = nc.declare_dram_parameter("recip", [P, SEG_PER_CORE // P], F32, isOutput=False)
    eyeb_h = nc.declare_dram_parameter("eye_bf16", [P, P], BF16, isOutput=False)
    eyef_h = nc.declare_dram_parameter("eye_f32", [P, P], F32, isOutput=False)
    ones_h = nc.declare_dram_parameter("ones_row", [1, P], F32, isOutput=False)
    w_h = nc.declare_dram_parameter("Wmat", [2 * C, C], F32, isOutput=False)
    b_h = nc.declare_dram_parameter("brow", [1, C], F32, isOutput=False)
    y_h = nc.declare_dram_parameter("y", [SEG_PER_CORE, C], F32, isOutput=True)

    with tile.TileContext(nc) as tc:
        with tc.tile_pool(name="persist", bufs=1) as pers, \
             tc.tile_pool(name="xs", bufs=3) as xs, \
             tc.tile_pool(name="fold", bufs=2) as fold, \
             tc.tile_pool(name="mrow", bufs=2) as mrow, \
             tc.tile_pool(name="cmb", bufs=1) as cmb, \
             tc.tile_pool(name="pss", bufs=2, space="PSUM") as pss, \
             tc.tile_pool(name="psm", bufs=2, space="PSUM") as psm, \
             tc.tile_pool(name="pbc", bufs=1, space="PSUM") as pbc:

            eyeb_t = pers.tile([P, P], BF16)
            nc.sync.dma_start(eyeb_t[:], eyeb_h[:])
            eyef_t = pers.tile([P, P], F32)
            nc.sync.dma_start(eyef_t[:], eyef_h[:])
            ones_t = pers.tile([1, P], F32)
            nc.sync.dma_start(ones_t[:], ones_h[:])
            onesb_t = pers.tile([1, P], BF16)
            nc.gpsimd.dma_start(onesb_t[:], ones_h[:])
            gidx_t = pers.tile([P, SEG_PER_CORE // 16], mybir.dt.uint16)
            nc.sync.dma_start(gidx_t[:], gi_h[:])
            recip_t = pers.tile([P, SEG_PER_CORE // P], F32)
            nc.sync.dma_start(recip_t[:], rc_h[:])
            wtop_t = pers.tile([P, C], F32)
            nc.sync.dma_start(wtop_t[:], w_h[0:C, :])
            wbot_t = pers.tile([P, C], F32)
            nc.sync.dma_start(wbot_t[:], w_h[C:2 * C, :])
            brow_t = pers.tile([1, C], F32)
            nc.sync.dma_start(brow_t[:], b_h[:])

            sumg = pers.tile([P, NG], F32)
            maxg = pers.tile([P, NG], BF16)

            # ---- phase A: stream node rows; fold maxes (DVE), sum via PE ----
            # Block-level software pipeline (1 block = 8192 rows = 4 fold
            # sets = 512 groups). At block b we emit: sum-matmuls(b),
            # sums-copy(b-1), DVE rounds r1(b)/r2(b-1)/r3(b-2)/r4(b-3) (all
            # mutually independent - no back-to-back dependent DVE ops),
            # then maxT-matmuls(b-3) + maxg-copy(b-3). Chunks (1024 groups =
            # 2 blocks) broadcast after their second block's maxg copy and
            # scan one block later.
            GPB = DBLK // GROUP  # groups per block (512)
            mask_tiles = {}

            def prefetch_masks(ch):
                off = ch * CHUNK
                n = min(CHUNK, NG - off)
                ma_t = mrow.tile([1, CHUNK], BF16, tag="ma")
                nc.sync.dma_start(ma_t[:, 0:n], ma_h[:, off:off + n])
                mm_t = mrow.tile([1, CHUNK], BF16, tag="mm")
                nc.sync.dma_start(mm_t[:, 0:n], mm_h[:, off:off + n])
                mask_tiles[ch] = (ma_t, mm_t)

            def emit_bcast(ch):
                off = ch * CHUNK
                n = min(CHUNK, NG - off)
                ma_t, mm_t = mask_tiles[ch]
                psb = pbc.tile([P, 2, CHUNK], F32, tag="bc")
                for j in range(0, n, 512):
                    w = min(512, n - j)
                    nc.tensor.matmul(psb[:, 0, j:j + w], onesb_t[:],
                                     ma_t[:, j:j + w], start=True, stop=True)
                    nc.tensor.matmul(psb[:, 1, j:j + w], onesb_t[:],
                                     mm_t[:, j:j + w], start=True, stop=True)
                mask_tiles[ch] = psb

            def emit_scans(ch):
                off = ch * CHUNK
                n = min(CHUNK, NG - off)
                psb = mask_tiles.pop(ch)
                init_a = 0.0 if ch == 0 else maxg[:, off - 1:off]
                init_m = 0.0 if ch == 0 else sumg[:, off - 1:off]
                nc.vector.tensor_tensor_scan(
                    maxg[:, off:off + n], psb[:, 0, 0:n], maxg[:, off:off + n],
                    init_a, mybir.AluOpType.add, mybir.AluOpType.max)
                nc.vector.tensor_tensor_scan(
                    sumg[:, off:off + n], psb[:, 1, 0:n], sumg[:, off:off + n],
                    init_m, mybir.AluOpType.mult, mybir.AluOpType.add)

            xts, t1s, t2s, t3s, t4s, pss_t = {}, {}, {}, {}, {}, {}
            scans_emitted = 0

            def round1(b):
                xt = xts[b]
                t1 = fold.tile([P, 4 * 1024], BF16, tag="f1")
                nc.vector.tensor_max(
                    t1[:].rearrange("p (s e) -> p s e", s=4),
                    xt[:].rearrange("p (s h e) -> p s h e", s=4, h=2)[:, :, 0, :],
                    xt[:].rearrange("p (s h e) -> p s h e", s=4, h=2)[:, :, 1, :])
                t1s[b] = t1

            def round2(b):
                t1 = t1s.pop(b)
                t2 = fold.tile([P, 4 * 512], BF16, tag="f2")
                nc.vector.tensor_max(
                    t2[:].rearrange("p (s e) -> p s e", s=4),
                    t1[:].rearrange("p (s h e) -> p s h e", s=4, h=2)[:, :, 0, :],
                    t1[:].rearrange("p (s h e) -> p s h e", s=4, h=2)[:, :, 1, :])
                t2s[b] = t2

            def round3(b):
                t2 = t2s.pop(b)
                t3 = fold.tile([P, 4 * 256], BF16, tag="f3")
                nc.vector.tensor_max(
                    t3[:].rearrange("p (s e) -> p s e", s=4),
                    t2[:].rearrange("p (s h e) -> p s h e", s=4, h=2)[:, :, 0, :],
                    t2[:].rearrange("p (s h e) -> p s h e", s=4, h=2)[:, :, 1, :])
                t3s[b] = t3

            def round4(b):
                t3 = t3s.pop(b)
                t4 = fold.tile([P, 4 * GPS], BF16, tag="f4")
                nc.vector.tensor_max(
                    t4[:].rearrange("p (s e) -> p s e", s=4),
                    t3[:].rearrange("p (s h e) -> p s h e", s=4, h=2)[:, :, 0, :],
                    t3[:].rearrange("p (s h e) -> p s h e", s=4, h=2)[:, :, 1, :])
                t4s[b] = t4

            def sums_copy(b):
                ps = pss_t.pop(b)
                nc.scalar.copy(
                    sumg[:, b * GPB:(b + 1) * GPB].rearrange(
                        "p (s g) -> p s g", s=4),
                    ps[:])

            def max_t_and_copy(b):
                nonlocal scans_emitted
                t4 = t4s.pop(b)
                pm = psm.tile([P, 4, GPS], F32, tag="mt")
                for s in range(SETS_PER_DMA):
                    nc.tensor.matmul(pm[:, s, :], t4[:, s * GPS:(s + 1) * GPS],
                                     eyeb_t[:], start=True, stop=True)
                nc.scalar.copy(
                    maxg[:, b * GPB:(b + 1) * GPB].rearrange(
                        "p (s g) -> p s g", s=4),
                    pm[:])
                if b % 2 == 1:
                    emit_bcast(b // 2)
                if b % 2 == 0 and b >= 2:
                    emit_scans(b // 2 - 1)
                    scans_emitted += 1

            for blk in range(NB):
                xt = xs.tile([P, DBLK // P * C], BF16, tag="xblk")
                nc.gpsimd.dma_start(xt[:], xp_h[blk * P:(blk + 1) * P, :])
                xts[blk] = xt
                if blk % 2 == 0 and blk // 2 < NCH:
                    prefetch_masks(blk // 2)
                ps = pss.tile([P, 4, GPS], F32, tag="sm")
                pss_t[blk] = ps
                for s in range(SETS_PER_DMA):
                    o = s * GROUP * C
                    for j in range(GROUP):
                        nc.tensor.matmul(ps[:, s, :],
                                         xt[:, o + j * C:o + (j + 1) * C],
                                         eyeb_t[:],
                                         start=(j == 0), stop=(j == GROUP - 1))
                if blk >= 1:
                    sums_copy(blk - 1)
                round1(blk)
                if blk >= 1:
                    round2(blk - 1)
                if blk >= 2:
                    round3(blk - 2)
                if blk >= 3:
                    round4(blk - 3)
                    max_t_and_copy(blk - 3)
                if blk >= 3:
                    xts.pop(blk - 3, None)

            # epilogue: drain the software pipeline
            sums_copy(NB - 1)
            for b in range(NB, NB + 3):
                if b - 1 in t1s:
                    round2(b - 1)
                if b - 2 in t2s:
                    round3(b - 2)
                if b - 3 in t3s:
                    round4(b - 3)
                    max_t_and_copy(b - 3)
            for ch in range(scans_emitted, NCH):
                if ch not in mask_tiles:
                    prefetch_masks(ch)
                if isinstance(mask_tiles[ch], tuple):
                    emit_bcast(ch)
                emit_scans(ch)

            # ---- phase C: gather segment ends ----
            segsum = pers.tile([P, SEG_PER_CORE], F32)
            segmax_b = pers.tile([P, SEG_PER_CORE], BF16)
            segmax = pers.tile([P, SEG_PER_CORE], F32)
            with tc.tile_critical():
                nc.gpsimd.indirect_copy(segsum[:], sumg[:], gidx_t[:], True)
            with tc.tile_critical():
                nc.gpsimd.indirect_copy(segmax_b[:], maxg[:], gidx_t[:], True)
            nc.scalar.copy(segmax[:], segmax_b[:])

            # ---- phase D: gated combine, 128 segments at a time ----
            for chn in range(SEG_PER_CORE // P):
                sl = slice(chn * P, (chn + 1) * P)
                pd = pss.tile([P, 4, GPS], F32, tag="sm")
                pd = pd[:].rearrange("p s g -> p (s g)")
                nc.tensor.matmul(pd[:, 0:C], segsum[:, sl], wtop_t[:],
                                 start=True, stop=True)
                t1 = cmb.tile([P, C], F32, tag="t1")
                # rows are segments: scale by 1/count -> mean @ W_top
                nc.scalar.mul(t1[:], pd[:, 0:C], recip_t[:, chn:chn + 1])
                nc.tensor.matmul(pd[:, C:2 * C], segmax[:, sl], wbot_t[:],
                                 start=True, stop=False)
                nc.tensor.matmul(pd[:, C:2 * C], ones_t[:], brow_t[:],
                                 start=False, stop=True)
                z = cmb.tile([P, C], F32, tag="z")
                nc.vector.tensor_add(z[:], t1[:], pd[:, C:2 * C])
                alpha = cmb.tile([P, C], F32, tag="alpha")
                nc.scalar.activation(alpha[:], z[:],
                                     mybir.ActivationFunctionType.Sigmoid)
                pd2 = psm.tile([P, 4, GPS], F32, tag="mt")
                pd2 = pd2[:].rearrange("p s g -> p (s g)")
                nc.tensor.transpose(pd2[:, 0:C], segsum[:, sl], eyef_t[:])
                mean_t = cmb.tile([P, C], F32, tag="mean")
                nc.scalar.mul(mean_t[:], pd2[:, 0:C], recip_t[:, chn:chn + 1])
                nc.tensor.transpose(pd2[:, C:2 * C], segmax[:, sl], eyef_t[:])
                max_t = cmb.tile([P, C], F32, tag="maxt")
                nc.scalar.copy(max_t[:], pd2[:, C:2 * C])
                d = cmb.tile([P, C], F32, tag="d")
                nc.vector.tensor_sub(d[:], mean_t[:], max_t[:])
                e = cmb.tile([P, C], F32, tag="e")
                nc.vector.tensor_mul(e[:], alpha[:], d[:])
                o = cmb.tile([P, C], F32, tag="o")
                nc.vector.tensor_add(o[:], e[:], max_t[:])
                nc.scalar.dma_start(y_h[sl, :], o[:])

    nc.finalize()
    _split_multi_waits(nc)
    return nc


def prepare(x, batch, W, b):
    x = np.ascontiguousarray(np.asarray(x, dtype=np.float32))
    batch = np.asarray(batch).astype(np.int64)
    W = np.asarray(W, dtype=np.float32)
    b = np.asarray(b, dtype=np.float32)
    N = x.shape[0]

    counts = np.bincount(batch, minlength=N_GRAPHS).astype(np.int64)
    row_off = np.zeros(N_GRAPHS + 1, np.int64)
    np.cumsum(counts, out=row_off[1:])

    # groups per segment (>=1 so empty segments yield exact zeros)
    ngs = np.maximum((counts + GROUP - 1) // GROUP, 1)    # [8192]
    ngs_core = ngs.reshape(CORES, SEG_PER_CORE)
    ng_needed = int(ngs_core.sum(axis=1).max())
    NG = ((ng_needed + 511) // 512) * 512   # scan/bcast/DMA in 512 multiples
    NPB = NG * GROUP                        # multiple of 8192

    x_bf = x.astype(NPBF16)
    eye_b = np.eye(P, dtype=NPBF16)
    eye_f = np.eye(P, dtype=np.float32)
    ones_row = np.ones((1, P), np.float32)
    brow = b.reshape(1, C)
    NB = NPB // DBLK

    in_maps = []
    for m in range(CORES):
        s0 = m * SEG_PER_CORE
        segs = slice(s0, s0 + SEG_PER_CORE)
        cnt = counts[segs]
        ng = ngs[segs]
        gstart = np.zeros(SEG_PER_CORE, np.int64)
        np.cumsum(ng[:-1], out=gstart[1:])
        r0, r1 = int(row_off[s0]), int(row_off[s0 + SEG_PER_CORE])

        xp = np.zeros((NPB, C), NPBF16)
        if r1 > r0:
            seg_local = batch[r0:r1] - s0
            within = np.arange(r1 - r0, dtype=np.int64) - row_off[s0 + seg_local] + r0
            dest = gstart[seg_local] * GROUP + within
            xp[dest] = x_bf[r0:r1]
        # swizzle: [NB, s(4), g(128), j(16), C] -> [NB, g, s, j, C]
        xp = xp.reshape(NB, SETS_PER_DMA, FSET // GROUP, GROUP, C)
        xp = np.ascontiguousarray(xp.transpose(0, 2, 1, 3, 4))
        xp = xp.reshape(NB * P, SETS_PER_DMA * GROUP * C)

        mask_add = np.zeros(NG, NPBF16)
        mask_mul = np.ones(NG, NPBF16)
        mask_add[gstart] = NPBF16(-1e30)
        mask_mul[gstart] = NPBF16(0.0)

        endg = (gstart + ng - 1).astype(np.uint16)
        wrapped = np.zeros((16, SEG_PER_CORE // 16), np.uint16)
        for j in range(SEG_PER_CORE):
            wrapped[j % 16, j // 16] = endg[j]
        gidx = np.tile(wrapped, (CORES, 1))

        recip = (1.0 / np.maximum(cnt, 1)).astype(np.float32)
        recip_t = recip.reshape(SEG_PER_CORE // P, P).T.copy()  # [128, 8]

        in_maps.append({
            "xp": xp,
            "mask_add": mask_add[None, :],
            "mask_mul": mask_mul[None, :],
            "gidx": gidx,
            "recip": recip_t,
            "eye_bf16": eye_b,
            "eye_f32": eye_f,
            "ones_row": ones_row,
            "Wmat": W,
            "brow": brow,
        })

    nc = _build_program(NPB, NG)
    return nc, in_maps


def kernel(x, batch, W, b):
    nc, in_maps = prepare(x, batch, W, b)
    last_err = None
    for _attempt in range(3):
        try:
            res = run_bass_kernel_spmd(nc, in_maps, list(range(CORES)))
            break
        except Exception as err:  # intermittent axon fetch flake
            last_err = err
    else:
        raise last_err

    out = np.concatenate([res.results[m]["y"] for m in range(CORES)], axis=0)
    return out.astype(np.float32)


if __name__ == "__main__":
    import jax
    import reference
    with jax.default_device(jax.devices("cpu")[0]):
        inputs = {k: np.asarray(v) for k, v in reference.setup_inputs().items()}
        expected = np.asarray(reference.reference(**{k: v for k, v in inputs.items()}))
    actual = kernel(**inputs)
    err = np.abs(actual - expected).max() / max(np.abs(expected).max(), 1e-9)
    rel = np.linalg.norm(actual - expected) / max(np.linalg.norm(expected), 1e-30)
    print("max-abs-normalized error:", err)
    print("Relative error:", rel)
